# revision 16
# baseline (speedup 1.0000x reference)
"""Trainium2 Bass kernel for nn_KPositiveLossCLIP (distributed CLIP k-positive loss).

Strategy (8 NeuronCores, data-parallel):
  - logits01 = normalize(out0) @ normalize(out1).T  -> row-sharded: core c computes
    rows [c*512, (c+1)*512) against all 4096 columns (out1 replicated).
  - Row sums of exp(logits/T) (for row log-softmax) via ACT accum during the exp pass.
  - Column sums of exp(logits/T) via PE matmuls with a ones vector (partial per core,
    summed on host) -> column log-softmax.
  - The [B,B] mask is never materialized: the loss only needs
    nnz[i] = sum_j sim[ind_i, ind_j] = (sim @ hist)[ind_i] where hist is the
    histogram of `indices`.  sim is row-sharded (32MB/core) and the mat-vec runs
    as a fused multiply+reduce on the vector engine while the sim rows stream in.
  - Host finishing: exact jax.random.randint replication (same jax call the
    reference makes), gather K entries/row from logits01, final scalar loss.
"""

from contextlib import ExitStack

import numpy as np

import concourse.bacc as bacc
import concourse.mybir as mybir
import concourse.tile as tile
from concourse.bass_utils import run_bass_kernel_spmd

B = 4096
D = 128
N = 8192
K = 6
TEMPERATURE = 0.1
NCORES = 8
RPC = B // NCORES    # logits rows per core (512)
SRPC = N // NCORES   # sim rows per core (1024)

f32 = mybir.dt.float32
bf16 = mybir.dt.bfloat16

_CACHE = {}


def _build():
    nc = bacc.Bacc("TRN2", target_bir_lowering=False, debug=False, num_devices=NCORES)
    a0t_d = nc.dram_tensor("a0t", [D, RPC], f32, kind="ExternalInput")
    a1t_d = nc.dram_tensor("a1t", [D, B], f32, kind="ExternalInput")
    sim_d = nc.dram_tensor("sim", [SRPC, N], f32, kind="ExternalInput")
    crep_d = nc.dram_tensor("crep", [128, N], f32, kind="ExternalInput")
    lg_d = nc.dram_tensor("logits", [RPC, B], f32, kind="ExternalOutput")
    rs_d = nc.dram_tensor("rowsum", [128, RPC // 128], f32, kind="ExternalOutput")
    cs_d = nc.dram_tensor("colsum", [128, B // 128], f32, kind="ExternalOutput")
    w_d = nc.dram_tensor("wv", [128, 2 * (SRPC // 128)], f32, kind="ExternalOutput")

    Exp = mybir.ActivationFunctionType.Exp
    Ln = mybir.ActivationFunctionType.Ln
    Copy = mybir.ActivationFunctionType.Copy
    X = mybir.AxisListType.X
    mult = mybir.AluOpType.mult
    add = mybir.AluOpType.add

    with tile.TileContext(nc) as tc, ExitStack() as ctx:
        const = ctx.enter_context(tc.tile_pool(name="const", bufs=1))
        big = ctx.enter_context(tc.tile_pool(name="big", bufs=2))
        gpool = ctx.enter_context(tc.tile_pool(name="gpool", bufs=2))
        spool = ctx.enter_context(tc.tile_pool(name="spool", bufs=2))
        small = ctx.enter_context(tc.tile_pool(name="small", bufs=3))
        expp = ctx.enter_context(tc.tile_pool(name="expp", bufs=3))
        stat = ctx.enter_context(tc.tile_pool(name="stat", bufs=1))
        psmm = ctx.enter_context(tc.tile_pool(name="psmm", bufs=3, space="PSUM"))
        pscs = ctx.enter_context(tc.tile_pool(name="pscs", bufs=1, space="PSUM"))
        pspr = ctx.enter_context(tc.tile_pool(name="pspr", bufs=2, space="PSUM"))

        ones = const.tile([128, 128], f32, tag="ones")
        nc.vector.memset(ones, 1.0)
        crep = const.tile([128, N], f32, tag="crep")
        nc.sync.dma_start(crep, crep_d.ap())
        a1t = const.tile([128, B], f32, tag="a1t")
        nc.sync.dma_start(a1t, a1t_d.ap())
        a0t = const.tile([128, RPC], f32, tag="a0t")
        nc.sync.dma_start(a0t, a0t_d.ap())

        w_sb = stat.tile([128, 2 * (SRPC // 128)], f32, tag="wsb")
        rs_sb = stat.tile([128, RPC // 128], f32, tag="rssb")
        cs_sb = stat.tile([128, B // 128], f32, tag="cssb")

        # ---- normalize a1t / a0t in place (columns = embedding rows) ----
        def normalize(tcol, width):
            for t in range(width // 512):
                sl = slice(t * 512, (t + 1) * 512)
                sq = small.tile([128, 512], f32, tag="sq")
                nc.vector.tensor_mul(sq, tcol[:, sl], tcol[:, sl])
                ps = pspr.tile([128, 512], f32, tag="pspr")
                nc.tensor.matmul(ps, ones, sq, start=True, stop=True)
                rn = small.tile([128, 512], f32, tag="rn")
                nc.scalar.activation(out=rn, in_=ps, func=Ln)
                nc.scalar.activation(out=rn, in_=rn, func=Exp, scale=-0.5)
                nc.vector.tensor_mul(tcol[:, sl], tcol[:, sl], rn)

        normalize(a1t, B)
        normalize(a0t, RPC)

        # ---- main loop: logits rows + row/col exp sums ----
        # Column sums accumulate into one PSUM bank across all 32 m-chunks with
        # start=False: memset gives the interp a zero base, and on HW the first
        # PE write per element lands with has_written=0 (overwrite), so the
        # accumulation is correct without start=True zero-region clears (which
        # would wipe sibling columns in the same 2KB zero region).
        psum_cs = pscs.tile([128, B // 128], f32, tag="pscs")
        nc.vector.memset(psum_cs, 0.0)
        for m in range(RPC // 128):
            lg = big.tile([128, B], f32, tag="lg")
            rsacc = small.tile([128, B // 512], f32, tag="rsacc")
            for n in range(B // 512):
                nsl = slice(n * 512, (n + 1) * 512)
                ps = psmm.tile([128, 512], f32, tag="psmm")
                nc.tensor.matmul(
                    ps, a0t[:, m * 128 : (m + 1) * 128], a1t[:, nsl],
                    start=True, stop=True,
                )
                if (m * 8 + n) % 2 == 0:
                    nc.vector.tensor_copy(lg[:, nsl], ps)
                else:
                    nc.scalar.copy(lg[:, nsl], ps)
                ex = expp.tile([128, 512], f32, tag="ex")
                nc.scalar.activation(
                    out=ex, in_=ps, func=Exp, scale=1.0 / TEMPERATURE,
                    accum_out=rsacc[:, n : n + 1],
                )
                for g in range(4):
                    col = n * 4 + g
                    nc.tensor.matmul(
                        psum_cs[:, col : col + 1],
                        ex[:, g * 128 : (g + 1) * 128],
                        ones[:, 0:1],
                        start=False, stop=False,
                        skip_group_check=True,
                    )
            nc.vector.reduce_sum(rs_sb[:, m : m + 1], rsacc, axis=X)
            nc.sync.dma_start(lg_d.ap()[m * 128 : (m + 1) * 128, :], lg)
        nc.vector.tensor_copy(cs_sb, psum_cs)
        nc.sync.dma_start(cs_d.ap(), cs_sb)
        nc.sync.dma_start(rs_d.ap(), rs_sb)

        # ---- nnz mat-vec: stream sim rows, fused mul+reduce against histogram.
        # Half-row chunks [128, 4096]; two partial accumulators per 128-row
        # group, summed on the host.
        H = N // 2
        for s in range(SRPC // 128):
            for h in range(2):
                g = gpool.tile([128, H], f32, tag="g")
                nc.scalar.dma_start(
                    g,
                    sim_d.ap()[s * 128 : (s + 1) * 128, h * H : (h + 1) * H],
                )
                prod = spool.tile([128, H], f32, tag="prod")
                nc.vector.tensor_mul(prod, g, crep[:, h * H : (h + 1) * H])
                scr = spool.tile([128, H], f32, tag="scr")
                nc.scalar.activation(
                    out=scr, in_=prod, func=Copy,
                    accum_out=w_sb[:, 2 * s + h : 2 * s + h + 1],
                )
        nc.scalar.dma_start(w_d.ap(), w_sb)

    nc.compile()
    return nc


def _get_nc():
    if "nc" not in _CACHE:
        _CACHE["nc"] = _build()
    return _CACHE["nc"]


def kernel(out0, out1, indices, sim_matrix):
    out0 = np.asarray(out0, dtype=np.float32)
    out1 = np.asarray(out1, dtype=np.float32)
    indices = np.asarray(indices, dtype=np.int32)
    sim_matrix = np.asarray(sim_matrix, dtype=np.float32)

    nc = _get_nc()

    hist = np.bincount(indices, minlength=N).astype(np.float32)
    crep = np.ascontiguousarray(np.broadcast_to(hist[None, :], (128, N)))
    a0t_full = np.ascontiguousarray(out0.T)
    a1t = np.ascontiguousarray(out1.T)

    in_maps = []
    for c in range(NCORES):
        in_maps.append(
            {
                "a0t": np.ascontiguousarray(a0t_full[:, c * RPC : (c + 1) * RPC]),
                "a1t": a1t,
                "sim": sim_matrix[c * SRPC : (c + 1) * SRPC],
                "crep": crep,
            }
        )

    res = run_bass_kernel_spmd(nc, in_maps, core_ids=list(range(NCORES)))

    logits01 = np.concatenate(
        [res.results[c]["logits"] for c in range(NCORES)], axis=0
    )
    rowsum = np.concatenate(
        [res.results[c]["rowsum"].T.reshape(-1) for c in range(NCORES)]
    )
    colsum = np.sum([res.results[c]["colsum"] for c in range(NCORES)], axis=0)
    colsum_flat = colsum.T.reshape(-1)
    w_pairs = np.concatenate(
        [
            res.results[c]["wv"].T.reshape(SRPC // 128, 2, 128).sum(axis=1).reshape(-1)
            for c in range(NCORES)
        ]
    )
    w = w_pairs

    nnz = np.rint(w[indices]).astype(np.int32)

    lse0 = np.log(rowsum.astype(np.float64))
    lse1 = np.log(colsum_flat.astype(np.float64))

    # exact replication of the reference's positive sampling
    import jax
    import jax.numpy as jnp

    rand_idx = np.asarray(
        jax.random.randint(
            jax.random.key(42), (B, K - 1), 0, jnp.asarray(nnz)[:, None]
        )
    )
    cols = np.concatenate(
        [np.arange(B, dtype=np.int64)[:, None], rand_idx.astype(np.int64)], axis=1
    )
    cols_sorted = np.sort(cols, axis=1)
    uniq = np.ones_like(cols_sorted, dtype=np.float64)
    uniq[:, 1:] = (cols_sorted[:, 1:] != cols_sorted[:, :-1]).astype(np.float64)
    cnt = uniq.sum(axis=1)

    rows = np.arange(B, dtype=np.int64)[:, None]
    v0 = logits01[rows, cols_sorted].astype(np.float64)
    v1 = logits01[cols_sorted, rows].astype(np.float64)
    s0 = (v0 * uniq).sum(axis=1) / TEMPERATURE - cnt * lse0
    s1 = (v1 * uniq).sum(axis=1) / TEMPERATURE - cnt * lse1
    total = s0.sum() + s1.sum()
    loss = -total / (B * (K - 1)) / (2 * B)

    return (
        np.float32(loss),
        logits01,
        np.arange(B, dtype=np.int32),
    )


# revision 18
# speedup vs baseline: 1.2538x; 1.2538x over previous
"""Trainium2 Bass kernel for nn_KPositiveLossCLIP (distributed CLIP k-positive loss).

Strategy (8 NeuronCores, data-parallel):
  - logits01 = normalize(out0) @ normalize(out1).T  -> row-sharded: core c computes
    rows [c*512, (c+1)*512) against all 4096 columns (out1 replicated).
  - Row sums of exp(logits/T) (for row log-softmax) via ACT accum during the exp pass.
  - Column sums of exp(logits/T) via PE matmuls with a ones vector (partial per core,
    summed on host) -> column log-softmax.
  - The [B,B] mask is never materialized: the loss only needs
    nnz[i] = sum_j sim[ind_i, ind_j] = (sim @ hist)[ind_i] where hist is the
    histogram of `indices`.  sim is row-sharded (32MB/core) and the mat-vec runs
    as a fused multiply+reduce on the vector engine while the sim rows stream in.
  - Host finishing: exact jax.random.randint replication (same jax call the
    reference makes), gather K entries/row from logits01, final scalar loss.
"""

from contextlib import ExitStack

import numpy as np

import concourse.bacc as bacc
import concourse.mybir as mybir
import concourse.tile as tile
from concourse.bass_utils import run_bass_kernel_spmd

B = 4096
D = 128
N = 8192
K = 6
TEMPERATURE = 0.1
NCORES = 8
RPC = B // NCORES    # logits rows per core (512)
SRPC = N // NCORES   # sim rows per core (1024)

f32 = mybir.dt.float32
bf16 = mybir.dt.bfloat16

_CACHE = {}


def _build():
    nc = bacc.Bacc("TRN2", target_bir_lowering=False, debug=False, num_devices=NCORES)
    a0t_d = nc.dram_tensor("a0t", [D, RPC], f32, kind="ExternalInput")
    a1t_d = nc.dram_tensor("a1t", [D, B], f32, kind="ExternalInput")
    sim_d = nc.dram_tensor("sim", [SRPC, N], f32, kind="ExternalInput")
    crep_d = nc.dram_tensor("crep", [128, N], f32, kind="ExternalInput")
    lg_d = nc.dram_tensor("logits", [RPC, B], f32, kind="ExternalOutput")
    rs_d = nc.dram_tensor("rowsum", [128, RPC // 128], f32, kind="ExternalOutput")
    cs_d = nc.dram_tensor("colsum", [128, B // 128], f32, kind="ExternalOutput")
    w_d = nc.dram_tensor("wv", [128, 2 * (SRPC // 128)], f32, kind="ExternalOutput")

    Exp = mybir.ActivationFunctionType.Exp
    Ln = mybir.ActivationFunctionType.Ln
    Copy = mybir.ActivationFunctionType.Copy
    X = mybir.AxisListType.X
    mult = mybir.AluOpType.mult
    add = mybir.AluOpType.add

    with tile.TileContext(nc) as tc, ExitStack() as ctx:
        const = ctx.enter_context(tc.tile_pool(name="const", bufs=1))
        big = ctx.enter_context(tc.tile_pool(name="big", bufs=2))
        gpool = ctx.enter_context(tc.tile_pool(name="gpool", bufs=2))
        spool = ctx.enter_context(tc.tile_pool(name="spool", bufs=2))
        small = ctx.enter_context(tc.tile_pool(name="small", bufs=3))
        expp = ctx.enter_context(tc.tile_pool(name="expp", bufs=3))
        stat = ctx.enter_context(tc.tile_pool(name="stat", bufs=1))
        psmm = ctx.enter_context(tc.tile_pool(name="psmm", bufs=3, space="PSUM"))
        pscs = ctx.enter_context(tc.tile_pool(name="pscs", bufs=1, space="PSUM"))
        pspr = ctx.enter_context(tc.tile_pool(name="pspr", bufs=2, space="PSUM"))

        ones = const.tile([128, 128], f32, tag="ones")
        nc.vector.memset(ones, 1.0)
        crep = const.tile([128, N], f32, tag="crep")
        nc.sync.dma_start(crep, crep_d.ap())
        a1t = const.tile([128, B], f32, tag="a1t")
        nc.sync.dma_start(a1t, a1t_d.ap())
        a0t = const.tile([128, RPC], f32, tag="a0t")
        nc.sync.dma_start(a0t, a0t_d.ap())

        w_sb = stat.tile([128, 2 * (SRPC // 128)], f32, tag="wsb")
        rs_sb = stat.tile([128, RPC // 128], f32, tag="rssb")
        cs_sb = stat.tile([128, B // 128], f32, tag="cssb")

        # ---- normalize a1t / a0t in place (columns = embedding rows) ----
        def normalize(tcol, width):
            for t in range(width // 512):
                sl = slice(t * 512, (t + 1) * 512)
                sq = small.tile([128, 512], f32, tag="sq")
                nc.vector.tensor_mul(sq, tcol[:, sl], tcol[:, sl])
                ps = pspr.tile([128, 512], f32, tag="pspr")
                nc.tensor.matmul(ps, ones, sq, start=True, stop=True)
                rn = small.tile([128, 512], f32, tag="rn")
                nc.scalar.activation(out=rn, in_=ps, func=Ln)
                nc.scalar.activation(out=rn, in_=rn, func=Exp, scale=-0.5)
                nc.vector.tensor_mul(tcol[:, sl], tcol[:, sl], rn)

        normalize(a1t, B)
        normalize(a0t, RPC)

        # ---- nnz mat-vec: stream sim rows, multiply by the histogram, row-sum.
        # Issued FIRST on the sync HWDGE ring (outputs go on the scalar ring) so
        # the 32MB stream starts immediately and overlaps the whole main loop.
        H = N // 2
        for s in range(SRPC // 128):
            for h in range(2):
                g = gpool.tile([128, H], f32, tag="g")
                nc.sync.dma_start(
                    g,
                    sim_d.ap()[s * 128 : (s + 1) * 128, h * H : (h + 1) * H],
                )
                prod = spool.tile([128, H], f32, tag="prod")
                nc.vector.tensor_mul(prod, g, crep[:, h * H : (h + 1) * H])
                scr = spool.tile([128, H], f32, tag="scr")
                nc.scalar.activation(
                    out=scr, in_=prod, func=Copy,
                    accum_out=w_sb[:, 2 * s + h : 2 * s + h + 1],
                )

        # ---- main loop: logits rows + row/col exp sums ----
        # Column sums accumulate into one PSUM bank across all 32 m-chunks with
        # start=False: memset gives the interp a zero base, and on HW the first
        # PE write per element lands with has_written=0 (overwrite), so the
        # accumulation is correct without start=True zero-region clears (which
        # would wipe sibling columns in the same 2KB zero region).
        psum_cs = pscs.tile([128, B // 128], f32, tag="pscs")
        nc.vector.memset(psum_cs, 0.0)
        for m in range(RPC // 128):
            lg = big.tile([128, B], f32, tag="lg")
            rsacc = small.tile([128, B // 512], f32, tag="rsacc")
            for n in range(B // 512):
                nsl = slice(n * 512, (n + 1) * 512)
                ps = psmm.tile([128, 512], f32, tag="psmm")
                nc.tensor.matmul(
                    ps, a0t[:, m * 128 : (m + 1) * 128], a1t[:, nsl],
                    start=True, stop=True,
                )
                if (m * 8 + n) % 2 == 0:
                    nc.vector.tensor_copy(lg[:, nsl], ps)
                else:
                    nc.scalar.copy(lg[:, nsl], ps)
                ex = expp.tile([128, 512], f32, tag="ex")
                nc.scalar.activation(
                    out=ex, in_=ps, func=Exp, scale=1.0 / TEMPERATURE,
                    accum_out=rsacc[:, n : n + 1],
                )
                for g in range(4):
                    col = n * 4 + g
                    nc.tensor.matmul(
                        psum_cs[:, col : col + 1],
                        ex[:, g * 128 : (g + 1) * 128],
                        ones[:, 0:1],
                        start=False, stop=False,
                        skip_group_check=True,
                    )
            nc.vector.reduce_sum(rs_sb[:, m : m + 1], rsacc, axis=X)
            nc.scalar.dma_start(lg_d.ap()[m * 128 : (m + 1) * 128, :], lg)
        nc.vector.tensor_copy(cs_sb, psum_cs)
        nc.scalar.dma_start(cs_d.ap(), cs_sb)
        nc.scalar.dma_start(rs_d.ap(), rs_sb)
        nc.sync.dma_start(w_d.ap(), w_sb)

    nc.compile()
    return nc


def _get_nc():
    if "nc" not in _CACHE:
        _CACHE["nc"] = _build()
    return _CACHE["nc"]


def kernel(out0, out1, indices, sim_matrix):
    out0 = np.asarray(out0, dtype=np.float32)
    out1 = np.asarray(out1, dtype=np.float32)
    indices = np.asarray(indices, dtype=np.int32)
    sim_matrix = np.asarray(sim_matrix, dtype=np.float32)

    nc = _get_nc()

    hist = np.bincount(indices, minlength=N).astype(np.float32)
    crep = np.ascontiguousarray(np.broadcast_to(hist[None, :], (128, N)))
    a0t_full = np.ascontiguousarray(out0.T)
    a1t = np.ascontiguousarray(out1.T)

    in_maps = []
    for c in range(NCORES):
        in_maps.append(
            {
                "a0t": np.ascontiguousarray(a0t_full[:, c * RPC : (c + 1) * RPC]),
                "a1t": a1t,
                "sim": sim_matrix[c * SRPC : (c + 1) * SRPC],
                "crep": crep,
            }
        )

    res = run_bass_kernel_spmd(nc, in_maps, core_ids=list(range(NCORES)))

    logits01 = np.concatenate(
        [res.results[c]["logits"] for c in range(NCORES)], axis=0
    )
    rowsum = np.concatenate(
        [res.results[c]["rowsum"].T.reshape(-1) for c in range(NCORES)]
    )
    colsum = np.sum([res.results[c]["colsum"] for c in range(NCORES)], axis=0)
    colsum_flat = colsum.T.reshape(-1)
    w_pairs = np.concatenate(
        [
            res.results[c]["wv"].T.reshape(SRPC // 128, 2, 128).sum(axis=1).reshape(-1)
            for c in range(NCORES)
        ]
    )
    w = w_pairs

    nnz = np.rint(w[indices]).astype(np.int32)

    lse0 = np.log(rowsum.astype(np.float64))
    lse1 = np.log(colsum_flat.astype(np.float64))

    # exact replication of the reference's positive sampling
    import jax
    import jax.numpy as jnp

    rand_idx = np.asarray(
        jax.random.randint(
            jax.random.key(42), (B, K - 1), 0, jnp.asarray(nnz)[:, None]
        )
    )
    cols = np.concatenate(
        [np.arange(B, dtype=np.int64)[:, None], rand_idx.astype(np.int64)], axis=1
    )
    cols_sorted = np.sort(cols, axis=1)
    uniq = np.ones_like(cols_sorted, dtype=np.float64)
    uniq[:, 1:] = (cols_sorted[:, 1:] != cols_sorted[:, :-1]).astype(np.float64)
    cnt = uniq.sum(axis=1)

    rows = np.arange(B, dtype=np.int64)[:, None]
    v0 = logits01[rows, cols_sorted].astype(np.float64)
    v1 = logits01[cols_sorted, rows].astype(np.float64)
    s0 = (v0 * uniq).sum(axis=1) / TEMPERATURE - cnt * lse0
    s1 = (v1 * uniq).sum(axis=1) / TEMPERATURE - cnt * lse1
    total = s0.sum() + s1.sum()
    loss = -total / (B * (K - 1)) / (2 * B)

    return (
        np.float32(loss),
        logits01,
        np.arange(B, dtype=np.int32),
    )
